# revision 18
# baseline (speedup 1.0000x reference)
import sys

sys.path.insert(0, "/opt/trn_rl_repo")

import numpy as np
from contextlib import ExitStack

# Problem constants (hardcoded per contract: kernel.py is self-contained).
B, S, D, O, M, E = 8, 2048, 768, 512, 1536, 8
T = S  # tokens per core (data-parallel over batch: 1 batch row per core)
P = 128
DT = D // P   # 6 d-tiles
MT = M // P   # 12 m-tiles
NT = T // P   # 16 token tiles per core
NCORES = 8

_CACHE = {}


def _build():
    import concourse.bass as bass
    import concourse.tile as tile
    from concourse import bacc, mybir
    from concourse.masks import make_identity

    f32 = mybir.dt.float32
    bf16 = mybir.dt.bfloat16
    AF = mybir.ActivationFunctionType
    ALU = mybir.AluOpType

    nc = bacc.Bacc("TRN2", target_bir_lowering=False, debug=False,
                   num_devices=NCORES)

    x_d = nc.dram_tensor("x", (T, D), f32, kind="ExternalInput").ap()
    wg_d = nc.dram_tensor("w_gate", (D, E), f32, kind="ExternalInput").ap()
    bi_d = nc.dram_tensor("bias_in", (E, D), f32, kind="ExternalInput").ap()
    win_d = nc.dram_tensor("W_in", (E, M, D), f32, kind="ExternalInput").ap()
    wout_d = nc.dram_tensor("W_out", (E, O, M), f32, kind="ExternalInput").ap()
    bo_d = nc.dram_tensor("b_out", (E, O), f32, kind="ExternalInput").ap()
    wsc_d = nc.dram_tensor("W_sc", (E, O, D), f32, kind="ExternalInput").ap()
    negc_d = nc.dram_tensor("neg_c", (E, M), f32, kind="ExternalInput").ap()
    out_d = nc.dram_tensor("out", (T, O), f32, kind="ExternalOutput").ap()

    with tile.TileContext(nc) as tc, ExitStack() as ctx:
        const = ctx.enter_context(tc.tile_pool(name="const", bufs=1))
        # f32 staging for all HBM loads (x + weights), recycled serially.
        ws = ctx.enter_context(tc.tile_pool(name="ws", bufs=2))
        # bf16 chunk staging between cast and DMA-transpose.
        wc = ctx.enter_context(tc.tile_pool(name="wc", bufs=12))
        wt = ctx.enter_context(tc.tile_pool(name="wt", bufs=2))
        hp = ctx.enter_context(tc.tile_pool(name="hp", bufs=1))
        comb = ctx.enter_context(tc.tile_pool(name="comb", bufs=2))
        pmm1 = ctx.enter_context(tc.tile_pool(name="pmm1", bufs=4, space="PSUM"))
        pmm2 = ctx.enter_context(tc.tile_pool(name="pmm2", bufs=3, space="PSUM"))
        ptr = ctx.enter_context(tc.tile_pool(name="ptr", bufs=1, space="PSUM"))

        ident = const.tile([P, P], bf16)
        make_identity(nc, ident)
        ident_f = const.tile([P, P], f32)
        make_identity(nc, ident_f)

        # ---- persistent SBUF tensors ----
        xT = const.tile([P, DT, T], bf16)        # x transposed: [d_in, d_out, t]
        acc = const.tile([P, NT, O], f32)        # output accumulator
        g_exp = const.tile([P, NT, E], f32)      # unnormalized softmax numerators
        g_bf = const.tile([P, NT, E], bf16)
        rinv = const.tile([P, NT], f32)          # 1 / sum_e exp
        gsum = const.tile([P, NT], f32)
        gTexp = const.tile([P, NT, P], bf16)     # gates transposed [e<=8, tt, t]
        wgate_f = const.tile([P, DT, E], f32)
        wgate_sb = const.tile([P, DT, E], bf16)
        negcT = const.tile([P, MT, E], f32)      # [m_in, m_out, e]
        bo_sb = const.tile([P, O], bf16)

        # ---- small inputs (SP queue; staged through the ws pool) ----
        nc.sync.dma_start(wgate_f, wg_d.rearrange("(po pi) e -> pi po e", pi=P))
        nc.vector.tensor_copy(wgate_sb, wgate_f)
        ns = ws.tile([P, 3072], f32, tag="ws", name="negc_s")
        nc.sync.dma_start(ns[:E, :M], negc_d)
        bs = ws.tile([P, 3072], f32, tag="ws", name="bo_s")
        nc.sync.dma_start(bs[:E, :O], bo_d)
        nc.vector.tensor_copy(bo_sb[:E, :], bs[:E, :O])
        for g in range(3):
            ptf = ptr.tile([P, 4, P], f32, tag="tr")
            for i in range(4):
                mt = g * 4 + i
                nc.tensor.transpose(ptf[:, i, :E], ns[:E, mt * P:(mt + 1) * P],
                                    ident_f[:E, :E])
            nc.vector.tensor_copy(negcT[:, g * 4:(g + 1) * 4, :], ptf[:, :4, :E])

        # ---- x loads: 4 batched [128, 3072] DMAs -> 16 bf16 chunk casts ----
        xbs = []

        def load_x(q):
            xs = ws.tile([P, 3072], f32, tag="ws", name="xs")
            nc.sync.dma_start(
                xs.rearrange("p (c d) -> p c d", c=4),
                x_d[q * 512:(q + 1) * 512, :].rearrange("(c p) d -> p c d", p=P))
            for i in range(4):
                xb = wc.tile([P, D], bf16, tag="wc", name="xb")
                nc.vector.tensor_copy(xb, xs[:, i * D:(i + 1) * D])
                xbs.append(xb)

        def emit_loads(e, which):
            """Batched f32 loads + bf16 slice-casts. Returns chunk list."""
            chunks = []

            def stage(src_ap, dsts):
                st = ws.tile([P, 3072], f32, tag="ws", name="wst")
                view = st.rearrange("p (c d) -> p c d", c=4)
                if len(src_ap.shape) == 4:
                    view = st.rearrange("p (c h d) -> p c h d", c=2, h=2)
                nc.sync.dma_start(view, src_ap)
                for i in range(4):
                    wb = wc.tile([P, D], bf16, tag="wc", name="wb")
                    nc.vector.tensor_copy(wb, st[:, i * D:(i + 1) * D])
                    chunks.append((wb, dsts[i]))

            if which == "win":
                winT = wt.tile([P, DT, M], bf16, tag="winT")
                for l3 in range(3):  # m-rows 4 at a time
                    r0 = l3 * 4
                    stage(win_d[e, r0 * P:(r0 + 4) * P, :].rearrange(
                        "(c p) d -> p c d", p=P),
                        [winT[:, :, (r0 + i) * P:(r0 + i + 1) * P]
                         for i in range(4)])
                return chunks, winT
            else:
                woutT = wt.tile([P, MT, O], bf16, tag="woutT")
                wscT = wt.tile([P, DT, O], bf16, tag="wscT")
                for l2 in range(2):
                    # W_out o-rows 2 at a time; chunk k = (o-row r0+k//2,
                    # m-half k%2)
                    r0 = l2 * 2
                    stage(wout_d[e, r0 * P:(r0 + 2) * P, :].rearrange(
                        "(c p) (h m) -> p c h m", p=P, h=2),
                        [woutT[:, (k % 2) * DT:(k % 2 + 1) * DT,
                               (r0 + k // 2) * P:(r0 + k // 2 + 1) * P]
                         for k in range(4)])
                stage(wsc_d[e].rearrange("(c p) d -> p c d", p=P),
                      [wscT[:, :, i * P:(i + 1) * P] for i in range(4)])
                return chunks, woutT, wscT

        def transpose_chunks(chunks):
            for wb, dst in chunks:
                nc.sync.dma_start_transpose(dst, wb)

        # Startup DMA schedule: x first half -> W_in(e0) -> x second half ->
        # W_out/W_sc(e0). The first 8 x chunks are transposed on the PE
        # (idle during startup anyway), keeping the DMA engines free for the
        # weight stream; chunks 8-15 use DMA transposes as before.
        load_x(0)
        load_x(1)
        win0 = emit_loads(0, "win")
        transpose_chunks(win0[0])
        load_x(2)
        load_x(3)
        for tt in range(8, NT):
            nc.sync.dma_start_transpose(xT[:, :, tt * P:(tt + 1) * P], xbs[tt])
        rest0 = emit_loads(0, "rest")
        transpose_chunks(rest0[0])

        def pe_transpose_x(tt):
            """xT[:, :, tt] = xbs[tt]^T via 6 PE transposes + 2 PSUM copies."""
            for h in range(2):
                ptx = ptr.tile([P, 4, P], bf16, tag="tr")
                n = 4 if h == 0 else 2
                for i in range(n):
                    dt_ = h * 4 + i
                    nc.tensor.transpose(ptx[:, i, :],
                                        xbs[tt][:, dt_ * P:(dt_ + 1) * P],
                                        ident)
                nc.vector.tensor_copy(
                    xT[:, h * 4:h * 4 + n, tt * P:(tt + 1) * P], ptx[:, :n, :])

        # ---- gating + expert compute building blocks ----
        def gate_group(tt):
            pg = pmm2.tile([P, O], f32, tag="mm2")
            for dt_ in range(DT):
                nc.tensor.matmul(pg[:, :E], xT[:, dt_, tt * P:(tt + 1) * P],
                                 wgate_sb[:, dt_, :],
                                 start=(dt_ == 0), stop=(dt_ == DT - 1))
            nc.scalar.activation(g_exp[:, tt, :], pg[:, :E], AF.Exp)

        hT = hp.tile([P, MT, T // 2], bf16)

        def mm1_half(e, winT, th):
            t0 = th * (T // 2)
            for mt in range(MT):
                for tq in range(2):
                    ph = pmm1.tile([P, O], f32, tag="mm1")
                    for dt_ in range(DT):
                        nc.tensor.matmul(
                            ph, winT[:, dt_, mt * P:(mt + 1) * P],
                            xT[:, dt_, t0 + tq * O:t0 + (tq + 1) * O],
                            start=(dt_ == 0), stop=(dt_ == DT - 1))
                    nc.scalar.activation(hT[:, mt, tq * O:(tq + 1) * O], ph,
                                         AF.Gelu, bias=negcT[:, mt, e:e + 1],
                                         scale=1.0)

        def mm2_half(e, woutT, wscT, th):
            for t8 in range(8):
                tg = th * 8 + t8
                po = pmm2.tile([P, O], f32, tag="mm2")
                for mt in range(MT):
                    nc.tensor.matmul(po, hT[:, mt, t8 * P:(t8 + 1) * P],
                                     woutT[:, mt, :],
                                     start=(mt == 0), stop=False)
                for dt_ in range(DT):
                    nc.tensor.matmul(po, xT[:, dt_, tg * P:(tg + 1) * P],
                                     wscT[:, dt_, :],
                                     start=False, stop=(dt_ == DT - 1))
                tmp = comb.tile([P, O], f32, tag="tmp")
                nc.vector.tensor_scalar(out=tmp, in0=po,
                                        scalar1=g_exp[:, tg, e:e + 1],
                                        scalar2=rinv[:, tg:tg + 1],
                                        op0=ALU.mult, op1=ALU.mult)
                nc.gpsimd.tensor_add(acc[:, tg, :], acc[:, tg, :], tmp)
                if e == E - 1:
                    nc.scalar.dma_start(out_d[tg * P:(tg + 1) * P, :],
                                        acc[:, tg, :])

        # PE-transpose + gating on the first token half, then expert-0 mm1 on
        # that half (fills the PE while the rest of x loads and softmax
        # completes).
        for tt in range(8):
            pe_transpose_x(tt)
        for tt in range(8):
            gate_group(tt)
        mm1_half(0, win0[1], 0)
        for tt in range(8, NT):
            gate_group(tt)
        del xbs

        nc.vector.tensor_reduce(gsum, g_exp, axis=mybir.AxisListType.X, op=ALU.add)
        nc.vector.reciprocal(rinv, gsum)
        nc.gpsimd.tensor_copy(g_bf, g_exp)

        # transpose gates ([128,8] blocks -> [8,128]) for the b_out init matmul
        for g in range(4):
            pt = ptr.tile([P, 4, P], bf16, tag="tr")
            for i in range(4):
                tt = g * 4 + i
                nc.tensor.transpose(pt[:E, i, :], g_bf[:, tt, :], ident)
            nc.vector.tensor_copy(gTexp[:E, g * 4:(g + 1) * 4, :], pt[:E, :4, :])

        # acc init: acc[t, o] = (g_exp[t, :] @ b_out) * rinv[t]
        for tt in range(NT):
            pb = pmm2.tile([P, O], f32, tag="mm2")
            nc.tensor.matmul(pb, gTexp[:E, tt, :], bo_sb[:E, :])
            nc.vector.tensor_scalar_mul(acc[:, tt, :], pb,
                                        scalar1=rinv[:, tt:tt + 1])

        # ---- expert pipeline ----
        winT, woutT, wscT = win0[1], rest0[1], rest0[2]

        for e in range(E):
            if e == 0:
                # mm1_half(0, th=0) was emitted during the gating phase.
                mm2_half(0, woutT, wscT, 0)
                nwin = emit_loads(1, "win")
                nrest = emit_loads(1, "rest")
                transpose_chunks(nwin[0])
                transpose_chunks(nrest[0])
            else:
                # Prefetch e+1: batched loads + casts + transposes on SP/DVE,
                # fully decoupled from this expert's compute queues.
                if e + 1 < E:
                    nwin = emit_loads(e + 1, "win")
                    nrest = emit_loads(e + 1, "rest")
                    transpose_chunks(nwin[0])
                    transpose_chunks(nrest[0])
                mm1_half(e, winT, 0)
                mm2_half(e, woutT, wscT, 0)
            mm1_half(e, winT, 1)
            mm2_half(e, woutT, wscT, 1)

            if e + 1 < E:
                winT, woutT, wscT = nwin[1], nrest[1], nrest[2]

    nc.compile()
    return nc


def _get_nc():
    if "nc" not in _CACHE:
        _CACHE["nc"] = _build()
    return _CACHE["nc"]


def kernel(x, w_gate, bias_in, W_in, W_out, b_out, W_sc):
    from concourse.bass_utils import run_bass_kernel_spmd

    nc = _get_nc()
    x = np.ascontiguousarray(np.asarray(x, dtype=np.float32))
    shared = {
        "w_gate": np.ascontiguousarray(np.asarray(w_gate, dtype=np.float32)),
        "bias_in": np.ascontiguousarray(np.asarray(bias_in, dtype=np.float32)),
        "W_in": np.ascontiguousarray(np.asarray(W_in, dtype=np.float32)),
        "W_out": np.ascontiguousarray(np.asarray(W_out, dtype=np.float32)),
        "b_out": np.ascontiguousarray(np.asarray(b_out, dtype=np.float32)),
        "W_sc": np.ascontiguousarray(np.asarray(W_sc, dtype=np.float32)),
        "neg_c": np.ascontiguousarray(
            -np.einsum("ed,emd->em", np.asarray(bias_in, np.float64),
                       np.asarray(W_in, np.float64)).astype(np.float32)),
    }
    in_maps = [{"x": x[i], **shared} for i in range(NCORES)]
    res = run_bass_kernel_spmd(nc, in_maps, core_ids=list(range(NCORES)))
    out = np.stack([res.results[i]["out"] for i in range(NCORES)], axis=0)
    return out.astype(np.float32)


# revision 19
# speedup vs baseline: 1.1060x; 1.1060x over previous
import sys

sys.path.insert(0, "/opt/trn_rl_repo")

import numpy as np
from contextlib import ExitStack

# Problem constants (hardcoded per contract: kernel.py is self-contained).
B, S, D, O, M, E = 8, 2048, 768, 512, 1536, 8
T = S  # tokens per core (data-parallel over batch: 1 batch row per core)
P = 128
DT = D // P   # 6 d-tiles
MT = M // P   # 12 m-tiles
NT = T // P   # 16 token tiles per core
NCORES = 8

_CACHE = {}


def _build():
    import concourse.bass as bass
    import concourse.tile as tile
    from concourse import bacc, mybir
    from concourse.masks import make_identity

    f32 = mybir.dt.float32
    bf16 = mybir.dt.bfloat16
    AF = mybir.ActivationFunctionType
    ALU = mybir.AluOpType

    nc = bacc.Bacc("TRN2", target_bir_lowering=False, debug=False,
                   num_devices=NCORES)

    x_d = nc.dram_tensor("x", (T, D), f32, kind="ExternalInput").ap()
    wg_d = nc.dram_tensor("w_gate", (D, E), f32, kind="ExternalInput").ap()
    bi_d = nc.dram_tensor("bias_in", (E, D), f32, kind="ExternalInput").ap()
    win_d = nc.dram_tensor("W_in", (E, M, D), f32, kind="ExternalInput").ap()
    wout_d = nc.dram_tensor("W_out", (E, O, M), f32, kind="ExternalInput").ap()
    bo_d = nc.dram_tensor("b_out", (E, O), f32, kind="ExternalInput").ap()
    wsc_d = nc.dram_tensor("W_sc", (E, O, D), f32, kind="ExternalInput").ap()
    negc_d = nc.dram_tensor("neg_c", (E, M), f32, kind="ExternalInput").ap()
    out_d = nc.dram_tensor("out", (T, O), f32, kind="ExternalOutput").ap()

    with tile.TileContext(nc) as tc, ExitStack() as ctx:
        const = ctx.enter_context(tc.tile_pool(name="const", bufs=1))
        # f32 staging for all HBM loads (x + weights), recycled serially.
        ws = ctx.enter_context(tc.tile_pool(name="ws", bufs=2))
        # bf16 chunk staging between cast and DMA-transpose.
        wc = ctx.enter_context(tc.tile_pool(name="wc", bufs=12))
        wt = ctx.enter_context(tc.tile_pool(name="wt", bufs=2))
        hp = ctx.enter_context(tc.tile_pool(name="hp", bufs=1))
        comb = ctx.enter_context(tc.tile_pool(name="comb", bufs=2))
        pmm1 = ctx.enter_context(tc.tile_pool(name="pmm1", bufs=4, space="PSUM"))
        pmm2 = ctx.enter_context(tc.tile_pool(name="pmm2", bufs=2, space="PSUM"))
        ptr = ctx.enter_context(tc.tile_pool(name="ptr", bufs=2, space="PSUM"))

        ident = const.tile([P, P], bf16)
        make_identity(nc, ident)
        ident_f = const.tile([P, P], f32)
        make_identity(nc, ident_f)

        # ---- persistent SBUF tensors ----
        xT = const.tile([P, DT, T], bf16)        # x transposed: [d_in, d_out, t]
        acc = const.tile([P, NT, O], f32)        # output accumulator
        g_exp = const.tile([P, NT, E], f32)      # unnormalized softmax numerators
        g_bf = const.tile([P, NT, E], bf16)
        rinv = const.tile([P, NT], f32)          # 1 / sum_e exp
        gsum = const.tile([P, NT], f32)
        gTexp = const.tile([P, NT, P], bf16)     # gates transposed [e<=8, tt, t]
        wgate_f = const.tile([P, DT, E], f32)
        wgate_sb = const.tile([P, DT, E], bf16)
        negcT = const.tile([P, MT, E], f32)      # [m_in, m_out, e]
        bo_sb = const.tile([P, O], bf16)

        # ---- small inputs (SP queue; staged through the ws pool) ----
        nc.sync.dma_start(wgate_f, wg_d.rearrange("(po pi) e -> pi po e", pi=P))
        nc.vector.tensor_copy(wgate_sb, wgate_f)
        ns = ws.tile([P, 3072], f32, tag="ws", name="negc_s")
        nc.sync.dma_start(ns[:E, :M], negc_d)
        bs = ws.tile([P, 3072], f32, tag="ws", name="bo_s")
        nc.sync.dma_start(bs[:E, :O], bo_d)
        nc.vector.tensor_copy(bo_sb[:E, :], bs[:E, :O])
        for g in range(3):
            ptf = ptr.tile([P, 4, P], f32, tag="tr")
            for i in range(4):
                mt = g * 4 + i
                nc.tensor.transpose(ptf[:, i, :E], ns[:E, mt * P:(mt + 1) * P],
                                    ident_f[:E, :E])
            nc.vector.tensor_copy(negcT[:, g * 4:(g + 1) * 4, :], ptf[:, :4, :E])

        # ---- x loads: 4 batched [128, 3072] DMAs -> 16 bf16 chunk casts ----
        xbs = []

        def load_x(q):
            xs = ws.tile([P, 3072], f32, tag="ws", name="xs")
            nc.sync.dma_start(
                xs.rearrange("p (c d) -> p c d", c=4),
                x_d[q * 512:(q + 1) * 512, :].rearrange("(c p) d -> p c d", p=P))
            for i in range(4):
                xb = wc.tile([P, D], bf16, tag="wc", name="xb")
                nc.vector.tensor_copy(xb, xs[:, i * D:(i + 1) * D])
                xbs.append(xb)

        def emit_loads(e, which):
            """Batched f32 loads + bf16 slice-casts. Returns chunk list."""
            chunks = []

            def stage(src_ap, dsts):
                st = ws.tile([P, 3072], f32, tag="ws", name="wst")
                view = st.rearrange("p (c d) -> p c d", c=4)
                if len(src_ap.shape) == 4:
                    view = st.rearrange("p (c h d) -> p c h d", c=2, h=2)
                nc.sync.dma_start(view, src_ap)
                for i in range(4):
                    wb = wc.tile([P, D], bf16, tag="wc", name="wb")
                    nc.vector.tensor_copy(wb, st[:, i * D:(i + 1) * D])
                    chunks.append((wb, dsts[i]))

            if which == "win":
                winT = wt.tile([P, DT, M], bf16, tag="winT")
                for l3 in range(3):  # m-rows 4 at a time
                    r0 = l3 * 4
                    stage(win_d[e, r0 * P:(r0 + 4) * P, :].rearrange(
                        "(c p) d -> p c d", p=P),
                        [winT[:, :, (r0 + i) * P:(r0 + i + 1) * P]
                         for i in range(4)])
                return chunks, winT
            else:
                woutT = wt.tile([P, MT, O], bf16, tag="woutT")
                wscT = wt.tile([P, DT, O], bf16, tag="wscT")
                for l2 in range(2):
                    # W_out o-rows 2 at a time; chunk k = (o-row r0+k//2,
                    # m-half k%2)
                    r0 = l2 * 2
                    stage(wout_d[e, r0 * P:(r0 + 2) * P, :].rearrange(
                        "(c p) (h m) -> p c h m", p=P, h=2),
                        [woutT[:, (k % 2) * DT:(k % 2 + 1) * DT,
                               (r0 + k // 2) * P:(r0 + k // 2 + 1) * P]
                         for k in range(4)])
                stage(wsc_d[e].rearrange("(c p) d -> p c d", p=P),
                      [wscT[:, :, i * P:(i + 1) * P] for i in range(4)])
                return chunks, woutT, wscT

        def transpose_chunks(chunks):
            for wb, dst in chunks:
                nc.sync.dma_start_transpose(dst, wb)

        # Startup DMA schedule: x first half -> W_in(e0) -> x second half ->
        # W_out/W_sc(e0). The first 8 x chunks are transposed on the PE
        # (idle during startup anyway), keeping the DMA engines free for the
        # weight stream; chunks 8-15 use DMA transposes as before.
        load_x(0)
        load_x(1)
        win0 = emit_loads(0, "win")
        transpose_chunks(win0[0])
        load_x(2)
        load_x(3)
        for tt in range(8, NT):
            nc.sync.dma_start_transpose(xT[:, :, tt * P:(tt + 1) * P], xbs[tt])
        rest0 = emit_loads(0, "rest")
        transpose_chunks(rest0[0])

        def pe_transpose_x(tt):
            """xT[:, :, tt] = xbs[tt]^T via 6 PE transposes + 2 PSUM copies."""
            for h in range(2):
                ptx = ptr.tile([P, 4, P], bf16, tag="tr")
                n = 4 if h == 0 else 2
                for i in range(n):
                    dt_ = h * 4 + i
                    nc.tensor.transpose(ptx[:, i, :],
                                        xbs[tt][:, dt_ * P:(dt_ + 1) * P],
                                        ident)
                nc.vector.tensor_copy(
                    xT[:, h * 4:h * 4 + n, tt * P:(tt + 1) * P], ptx[:, :n, :])

        # ---- gating + expert compute building blocks ----
        def gate_group(tt):
            pg = pmm2.tile([P, O], f32, tag="mm2")
            for dt_ in range(DT):
                nc.tensor.matmul(pg[:, :E], xT[:, dt_, tt * P:(tt + 1) * P],
                                 wgate_sb[:, dt_, :],
                                 start=(dt_ == 0), stop=(dt_ == DT - 1))
            nc.scalar.activation(g_exp[:, tt, :], pg[:, :E], AF.Exp)

        hT = hp.tile([P, MT, T // 2], bf16)

        def mm1_half(e, winT, th):
            t0 = th * (T // 2)
            for mt in range(MT):
                for tq in range(2):
                    ph = pmm1.tile([P, O], f32, tag="mm1")
                    for dt_ in range(DT):
                        nc.tensor.matmul(
                            ph, winT[:, dt_, mt * P:(mt + 1) * P],
                            xT[:, dt_, t0 + tq * O:t0 + (tq + 1) * O],
                            start=(dt_ == 0), stop=(dt_ == DT - 1))
                    nc.scalar.activation(hT[:, mt, tq * O:(tq + 1) * O], ph,
                                         AF.Gelu, bias=negcT[:, mt, e:e + 1],
                                         scale=1.0)

        def mm2_half(e, woutT, wscT, th):
            for t8 in range(8):
                tg = th * 8 + t8
                po = pmm2.tile([P, O], f32, tag="mm2")
                for mt in range(MT):
                    nc.tensor.matmul(po, hT[:, mt, t8 * P:(t8 + 1) * P],
                                     woutT[:, mt, :],
                                     start=(mt == 0), stop=False)
                for dt_ in range(DT):
                    nc.tensor.matmul(po, xT[:, dt_, tg * P:(tg + 1) * P],
                                     wscT[:, dt_, :],
                                     start=False, stop=(dt_ == DT - 1))
                tmp = comb.tile([P, O], f32, tag="tmp")
                nc.vector.tensor_scalar(out=tmp, in0=po,
                                        scalar1=g_exp[:, tg, e:e + 1],
                                        scalar2=rinv[:, tg:tg + 1],
                                        op0=ALU.mult, op1=ALU.mult)
                nc.gpsimd.tensor_add(acc[:, tg, :], acc[:, tg, :], tmp)
                if e == E - 1:
                    nc.scalar.dma_start(out_d[tg * P:(tg + 1) * P, :],
                                        acc[:, tg, :])

        # PE-transpose + gating on the first token half, then expert-0 mm1 on
        # that half (fills the PE while the rest of x loads and softmax
        # completes).
        for tt in range(8):
            pe_transpose_x(tt)
        for tt in range(8):
            gate_group(tt)
        mm1_half(0, win0[1], 0)
        for tt in range(8, NT):
            gate_group(tt)
        del xbs

        nc.vector.tensor_reduce(gsum, g_exp, axis=mybir.AxisListType.X, op=ALU.add)
        nc.vector.reciprocal(rinv, gsum)
        nc.gpsimd.tensor_copy(g_bf, g_exp)

        # transpose gates ([128,8] blocks -> [8,128]) for the b_out init matmul
        for g in range(4):
            pt = ptr.tile([P, 4, P], bf16, tag="tr")
            for i in range(4):
                tt = g * 4 + i
                nc.tensor.transpose(pt[:E, i, :], g_bf[:, tt, :], ident)
            nc.vector.tensor_copy(gTexp[:E, g * 4:(g + 1) * 4, :], pt[:E, :4, :])

        # acc init: acc[t, o] = (g_exp[t, :] @ b_out) * rinv[t]
        for tt in range(NT):
            pb = pmm2.tile([P, O], f32, tag="mm2")
            nc.tensor.matmul(pb, gTexp[:E, tt, :], bo_sb[:E, :])
            nc.vector.tensor_scalar_mul(acc[:, tt, :], pb,
                                        scalar1=rinv[:, tt:tt + 1])

        # ---- expert pipeline ----
        winT, woutT, wscT = win0[1], rest0[1], rest0[2]

        for e in range(E):
            if e == 0:
                # mm1_half(0, th=0) was emitted during the gating phase.
                mm2_half(0, woutT, wscT, 0)
                nwin = emit_loads(1, "win")
                nrest = emit_loads(1, "rest")
                transpose_chunks(nwin[0])
                transpose_chunks(nrest[0])
            else:
                # Prefetch e+1: batched loads + casts + transposes on SP/DVE,
                # fully decoupled from this expert's compute queues.
                if e + 1 < E:
                    nwin = emit_loads(e + 1, "win")
                    nrest = emit_loads(e + 1, "rest")
                    transpose_chunks(nwin[0])
                    transpose_chunks(nrest[0])
                mm1_half(e, winT, 0)
                mm2_half(e, woutT, wscT, 0)
            mm1_half(e, winT, 1)
            mm2_half(e, woutT, wscT, 1)

            if e + 1 < E:
                winT, woutT, wscT = nwin[1], nrest[1], nrest[2]

    nc.compile()
    return nc


def _get_nc():
    if "nc" not in _CACHE:
        _CACHE["nc"] = _build()
    return _CACHE["nc"]


def kernel(x, w_gate, bias_in, W_in, W_out, b_out, W_sc):
    from concourse.bass_utils import run_bass_kernel_spmd

    nc = _get_nc()
    x = np.ascontiguousarray(np.asarray(x, dtype=np.float32))
    shared = {
        "w_gate": np.ascontiguousarray(np.asarray(w_gate, dtype=np.float32)),
        "bias_in": np.ascontiguousarray(np.asarray(bias_in, dtype=np.float32)),
        "W_in": np.ascontiguousarray(np.asarray(W_in, dtype=np.float32)),
        "W_out": np.ascontiguousarray(np.asarray(W_out, dtype=np.float32)),
        "b_out": np.ascontiguousarray(np.asarray(b_out, dtype=np.float32)),
        "W_sc": np.ascontiguousarray(np.asarray(W_sc, dtype=np.float32)),
        "neg_c": np.ascontiguousarray(
            -np.einsum("ed,emd->em", np.asarray(bias_in, np.float64),
                       np.asarray(W_in, np.float64)).astype(np.float32)),
    }
    in_maps = [{"x": x[i], **shared} for i in range(NCORES)]
    res = run_bass_kernel_spmd(nc, in_maps, core_ids=list(range(NCORES)))
    out = np.stack([res.results[i]["out"] for i in range(NCORES)], axis=0)
    return out.astype(np.float32)
